# revision 43
# baseline (speedup 1.0000x reference)
"""AttentionCropLayer Trainium2 kernel.

Per sample b: offsets (w,h) = floor(clip(locs[b]*224, 44, 180) - 44); output
out[b] = images[b, :, w:w+88, h:h+88] * mask, with mask the fixed 88x88
sigmoid-profile outer product.

Strategy (pure data parallel, 8 cores x 16 samples):
  - host precomputes per-sample flat element offsets (exact same f32 ops as
    the reference) plus small constant tables for the mask
  - device: one SBUF tile [128, 968] per sample with partition p = g*16 + c
    (g = crop-row group i//11, c = channel). Each of 11 dynamic-offset reads
    per sample spans all 128 partitions (measured: 128-partition DMAs run
    ~2-3x faster per SDMA engine than narrow ones) and moves exactly the
    needed 352B crop rows. One [128, 968] DVE multiply applies the mask
    (replicated on-chip via a tiny selection matmul); one static write per
    sample stores 968-element contiguous runs per partition.
"""

import sys

if "/opt/trn_rl_repo" not in sys.path:
    sys.path.insert(0, "/opt/trn_rl_repo")

import numpy as np

import concourse.bass as bass
import concourse.bacc as bacc
import concourse.mybir as mybir
from concourse import tile
from concourse.bass_utils import run_bass_kernel_spmd

TL = 44
CROP = 2 * TL          # 88
SCALE = 224.0
B, C, IN = 128, 16, 224
NCORES = 8
BPC = B // NCORES      # 16 samples per core
NG = 8                 # row groups per sample -> NG*C = 128 partitions
GR = CROP // NG        # 11 crop rows per group = reads per sample
PFREE = GR * CROP      # 968 elements per partition
FREE = CROP * CROP     # 7744
MAXOFF = IN - CROP     # 136
IMSZ = C * IN * IN
CHSZ = IN * IN
MAXEOFF = (BPC - 1) * IMSZ + (MAXOFF + GR - 1) * IN + MAXOFF

_nc_cache = {}


def _build_nc():
    nc = bacc.Bacc(None)
    images = nc.declare_dram_parameter(
        "images", [BPC, C, IN, IN], mybir.dt.float32, isOutput=False
    )
    offs = nc.declare_dram_parameter(
        "offs", [1, BPC], mybir.dt.int32, isOutput=False
    )
    maskrows = nc.declare_dram_parameter(
        "maskrows", [NG, PFREE], mybir.dt.float32, isOutput=False
    )
    sel8 = nc.declare_dram_parameter(
        "sel8", [NG, NG * C], mybir.dt.float32, isOutput=False
    )
    out = nc.declare_dram_parameter(
        "out", [BPC, C, CROP, CROP], mybir.dt.float32, isOutput=True
    )

    with tile.TileContext(nc) as tc:
        with (
            tc.tile_pool(name="const", bufs=1) as cpool,
            tc.tile_pool(name="work", bufs=6) as wpool,
            tc.tile_pool(name="psum", bufs=2, space="PSUM") as ppool,
        ):
            offs_sb = cpool.tile([1, BPC], mybir.dt.int32)
            nc.sync.dma_start(out=offs_sb[:], in_=offs[:])
            # mask_sb[p = g*16+c, r*88+k] = prof[11g+r]*prof[k]: replicate the
            # 8 per-group mask rows across channels with one selection matmul
            mrow_sb = cpool.tile([NG, PFREE], mybir.dt.float32)
            nc.gpsimd.dma_start(out=mrow_sb[:], in_=maskrows[:])
            sel_sb = cpool.tile([NG, NG * C], mybir.dt.float32)
            nc.gpsimd.dma_start(out=sel_sb[:], in_=sel8[:])
            mask_sb = cpool.tile([NG * C, PFREE], mybir.dt.float32)
            pcol = 484
            for ci in range((PFREE + pcol - 1) // pcol):
                lo = ci * pcol
                w = min(pcol, PFREE - lo)
                pt = ppool.tile([NG * C, pcol], mybir.dt.float32, tag="pmask")
                nc.tensor.matmul(
                    out=pt[:, 0:w],
                    lhsT=sel_sb[:],
                    rhs=mrow_sb[:, lo : lo + w],
                    start=True,
                    stop=True,
                )
                nc.vector.tensor_copy(out=mask_sb[:, lo : lo + w], in_=pt[:, 0:w])

            regs = {
                "sync": nc.sync.alloc_register("o_reg_sp"),
                "scalar": nc.scalar.alloc_register("o_reg_act"),
            }
            engs = {"sync": nc.sync, "scalar": nc.scalar}
            for s in range(BPC):
                rk = "sync" if s % 2 == 0 else "scalar"
                eng_, reg_ = engs[rk], regs[rk]
                t = wpool.tile([NG * C, PFREE], mybir.dt.float32, tag="blk")
                eng_.reg_load(reg_, offs_sb[0:1, s : s + 1])
                ov = eng_.snap(reg_, donate=True, min_val=0, max_val=MAXEOFF)
                base = images[s, :, 0:CROP, 0:CROP]
                for r in range(GR):
                    src = bass.AP(
                        tensor=base.tensor,
                        offset=ov + r * IN,
                        ap=[[GR * IN, NG], [CHSZ, C], [1, CROP]],
                        dep_tracking_offset=s * IMSZ,
                    )
                    eng_.dma_start(
                        out=t[:, r * CROP : (r + 1) * CROP], in_=src
                    )
                nc.vector.tensor_tensor(
                    out=t[:], in0=t[:], in1=mask_sb[:], op=mybir.AluOpType.mult
                )
                dst = bass.AP(
                    tensor=out[:].tensor,
                    offset=s * C * FREE,
                    ap=[[GR * CROP, NG], [FREE, C], [1, PFREE]],
                )
                nc.gpsimd.dma_start(out=dst, in_=t[:])
    nc.finalize()
    return nc


def _get_nc():
    if "nc" not in _nc_cache:
        _nc_cache["nc"] = _build_nc()
    return _nc_cache["nc"]


def _host_offsets(locs):
    locs = np.asarray(locs, dtype=np.float32)
    t = np.clip(locs * np.float32(SCALE), np.float32(TL), np.float32(IN - TL))
    return np.floor(t - np.float32(TL)).astype(np.int32)  # [B, 2] (w, h)


def _host_mask_tables():
    rr = np.arange(CROP, dtype=np.float32)
    sig = lambda z: (1.0 / (1.0 + np.exp(-10.0 * z, dtype=np.float32))).astype(
        np.float32
    )
    prof = sig(rr) - sig(rr - np.float32(CROP))
    mask = np.outer(prof, prof).astype(np.float32)  # [88, 88]
    # maskrows[g, r*88+k] = mask[11g+r, k]
    maskrows = mask.reshape(NG, GR * CROP).copy()
    # sel8[g, p] = 1 where p // 16 == g
    sel = np.zeros((NG, NG * C), dtype=np.float32)
    for g in range(NG):
        sel[g, g * C : (g + 1) * C] = 1.0
    return np.ascontiguousarray(maskrows), np.ascontiguousarray(sel)


def make_in_maps(images, locs):
    images = np.asarray(images, dtype=np.float32)
    off = _host_offsets(locs)  # [B, 2] (w, h)
    s_idx = np.arange(BPC, dtype=np.int64)
    maskrows, sel8 = _host_mask_tables()
    in_maps = []
    for i in range(NCORES):
        sl = slice(i * BPC, (i + 1) * BPC)
        osh = off[sl].astype(np.int64)
        eoff = (s_idx * IMSZ + osh[:, 0] * IN + osh[:, 1]).astype(np.int32)
        in_maps.append(
            {
                "images": np.ascontiguousarray(images[sl]),
                "offs": np.ascontiguousarray(eoff.reshape(1, -1)),
                "maskrows": maskrows,
                "sel8": sel8,
            }
        )
    return in_maps


def run(images, locs, trace=False, **kwargs):
    nc = _get_nc()
    in_maps = make_in_maps(images, locs)
    res = run_bass_kernel_spmd(
        nc, in_maps, core_ids=list(range(NCORES)), trace=trace, **kwargs
    )
    outs = [np.asarray(res.results[i]["out"]) for i in range(NCORES)]
    full = np.concatenate(outs, axis=0).astype(np.float32)
    return full, res


def kernel(images, locs):
    full, _ = run(images, locs, trace=False)
    return full


# revision 44
# speedup vs baseline: 1.1377x; 1.1377x over previous
"""AttentionCropLayer Trainium2 kernel.

Per sample b: offsets (w,h) = floor(clip(locs[b]*224, 44, 180) - 44); output
out[b] = images[b, :, w:w+88, h:h+88] * mask, with mask the fixed 88x88
sigmoid-profile outer product.

Strategy (pure data parallel, 8 cores x 16 samples):
  - host precomputes per-sample flat element offsets (exact same f32 ops as
    the reference) plus tiny constant tables for the mask
  - device, per chunk of 8 samples (=128 partitions, partition = (sample,
    channel)): per sample two dynamic-offset HWDGE read DMAs (channel halves,
    one per HWDGE ring so every SDMA engine interleaves two descriptor
    streams), each descriptor an exact 352B crop row; per chunk one in-place
    DVE mask-multiply (mask replicated on-chip via an outer-product matmul)
    and one contiguous 3.96MB store
"""

import sys

if "/opt/trn_rl_repo" not in sys.path:
    sys.path.insert(0, "/opt/trn_rl_repo")

import numpy as np

import concourse.bass as bass
import concourse.bacc as bacc
import concourse.mybir as mybir
from concourse import tile
from concourse.bass_utils import run_bass_kernel_spmd

TL = 44
CROP = 2 * TL          # 88
SCALE = 224.0
B, C, IN = 128, 16, 224
NCORES = 8
BPC = B // NCORES      # 16 samples per core
BLK = 8                # samples per chunk -> BLK*C = 128 partitions
NBLK = BPC // BLK      # 2 chunks per core
FREE = CROP * CROP     # 7744
HC = C // 2            # channel half
MAXOFF = IN - CROP     # 136
IMSZ = C * IN * IN
CHSZ = IN * IN
MAXEOFF = (BPC - 1) * IMSZ + MAXOFF * IN + MAXOFF

_nc_cache = {}


def _build_nc():
    nc = bacc.Bacc(None)
    images = nc.declare_dram_parameter(
        "images", [BPC, C, IN, IN], mybir.dt.float32, isOutput=False
    )
    offs = nc.declare_dram_parameter(
        "offs", [1, BPC], mybir.dt.int32, isOutput=False
    )
    maskrow = nc.declare_dram_parameter(
        "maskrow", [1, FREE], mybir.dt.float32, isOutput=False
    )
    ones1 = nc.declare_dram_parameter(
        "ones1", [1, BLK * C], mybir.dt.float32, isOutput=False
    )
    out = nc.declare_dram_parameter(
        "out", [BPC, C, CROP, CROP], mybir.dt.float32, isOutput=True
    )

    with tile.TileContext(nc) as tc:
        with (
            tc.tile_pool(name="const", bufs=1) as cpool,
            tc.tile_pool(name="work", bufs=2) as wpool,
            tc.tile_pool(name="psum", bufs=2, space="PSUM") as ppool,
        ):
            offs_sb = cpool.tile([1, BPC], mybir.dt.int32)
            nc.sync.dma_start(out=offs_sb[:], in_=offs[:])
            # replicate the [1, 7744] mask row to all 128 partitions on-chip
            mrow_sb = cpool.tile([1, FREE], mybir.dt.float32)
            nc.gpsimd.dma_start(out=mrow_sb[:], in_=maskrow[:])
            ones_sb = cpool.tile([1, BLK * C], mybir.dt.float32)
            nc.gpsimd.dma_start(out=ones_sb[:], in_=ones1[:])
            mask_sb = cpool.tile([BLK * C, FREE], mybir.dt.float32)
            pcol = 512
            for ci in range((FREE + pcol - 1) // pcol):
                lo = ci * pcol
                w = min(pcol, FREE - lo)
                pt = ppool.tile([BLK * C, pcol], mybir.dt.float32, tag="pmask")
                nc.tensor.matmul(
                    out=pt[:, 0:w],
                    lhsT=ones_sb[0:1, :],
                    rhs=mrow_sb[0:1, lo : lo + w],
                    start=True,
                    stop=True,
                )
                nc.vector.tensor_copy(out=mask_sb[:, lo : lo + w], in_=pt[:, 0:w])

            regs = {
                "sync": nc.sync.alloc_register("o_reg_sp"),
                "scalar": nc.scalar.alloc_register("o_reg_act"),
            }
            engs = {"sync": nc.sync, "scalar": nc.scalar}
            for blk in range(NBLK):
                t = wpool.tile([BLK * C, FREE], mybir.dt.float32, tag="blk")
                for j in range(BLK):
                    s = blk * BLK + j
                    base = images[s, :, 0:CROP, 0:CROP]
                    for hi, rk in enumerate(("sync", "scalar")):
                        eng_, reg_ = engs[rk], regs[rk]
                        eng_.reg_load(reg_, offs_sb[0:1, s : s + 1])
                        ov = eng_.snap(
                            reg_, donate=True, min_val=0, max_val=MAXEOFF
                        )
                        src = bass.AP(
                            tensor=base.tensor,
                            offset=ov + hi * HC * CHSZ,
                            ap=[[CHSZ, HC], [IN, CROP], [1, CROP]],
                            dep_tracking_offset=s * IMSZ + hi * HC * CHSZ,
                        )
                        p0 = j * C + hi * HC
                        eng_.dma_start(out=t[p0 : p0 + HC, :], in_=src)
                nc.vector.tensor_tensor(
                    out=t[:], in0=t[:], in1=mask_sb[:], op=mybir.AluOpType.mult
                )
                out_view = out[blk * BLK : (blk + 1) * BLK].rearrange(
                    "b c i k -> (b c) (i k)"
                )
                nc.gpsimd.dma_start(out=out_view, in_=t[:])
    nc.finalize()
    return nc


def _get_nc():
    if "nc" not in _nc_cache:
        _nc_cache["nc"] = _build_nc()
    return _nc_cache["nc"]


def _host_offsets(locs):
    locs = np.asarray(locs, dtype=np.float32)
    t = np.clip(locs * np.float32(SCALE), np.float32(TL), np.float32(IN - TL))
    return np.floor(t - np.float32(TL)).astype(np.int32)  # [B, 2] (w, h)


def _host_mask():
    rr = np.arange(CROP, dtype=np.float32)
    sig = lambda z: (1.0 / (1.0 + np.exp(-10.0 * z, dtype=np.float32))).astype(
        np.float32
    )
    prof = sig(rr) - sig(rr - np.float32(CROP))
    mask = np.outer(prof, prof).astype(np.float32).reshape(1, -1)
    return np.ascontiguousarray(mask)


def make_in_maps(images, locs):
    images = np.asarray(images, dtype=np.float32)
    off = _host_offsets(locs)  # [B, 2] (w, h)
    s_idx = np.arange(BPC, dtype=np.int64)
    maskrow = _host_mask()
    ones1 = np.ones((1, BLK * C), dtype=np.float32)
    in_maps = []
    for i in range(NCORES):
        sl = slice(i * BPC, (i + 1) * BPC)
        osh = off[sl].astype(np.int64)
        eoff = (s_idx * IMSZ + osh[:, 0] * IN + osh[:, 1]).astype(np.int32)
        in_maps.append(
            {
                "images": np.ascontiguousarray(images[sl]),
                "offs": np.ascontiguousarray(eoff.reshape(1, -1)),
                "maskrow": maskrow,
                "ones1": ones1,
            }
        )
    return in_maps


def run(images, locs, trace=False, **kwargs):
    nc = _get_nc()
    in_maps = make_in_maps(images, locs)
    res = run_bass_kernel_spmd(
        nc, in_maps, core_ids=list(range(NCORES)), trace=trace, **kwargs
    )
    outs = [np.asarray(res.results[i]["out"]) for i in range(NCORES)]
    full = np.concatenate(outs, axis=0).astype(np.float32)
    return full, res


def kernel(images, locs):
    full, _ = run(images, locs, trace=False)
    return full


# revision 45
# speedup vs baseline: 1.4338x; 1.2603x over previous
"""AttentionCropLayer Trainium2 kernel.

Per sample b: offsets (w,h) = floor(clip(locs[b]*224, 44, 180) - 44); output
out[b] = images[b, :, w:w+88, h:h+88] * mask, with mask the fixed 88x88
sigmoid-profile outer product.

Strategy (pure data parallel, 8 cores x 16 samples):
  - host precomputes per-sample flat element offsets (exact same f32 ops as
    the reference) plus tiny constant tables for the mask
  - device, per chunk of 8 samples (=128 partitions, partition = (sample,
    channel)): per sample two dynamic-offset HWDGE read DMAs (channel halves,
    one per HWDGE ring so every SDMA engine interleaves two descriptor
    streams), each descriptor an exact 352B crop row; per chunk one in-place
    DVE mask-multiply (mask replicated on-chip via an outer-product matmul)
    and one contiguous 3.96MB store
"""

import sys

if "/opt/trn_rl_repo" not in sys.path:
    sys.path.insert(0, "/opt/trn_rl_repo")

import numpy as np

import concourse.bass as bass
import concourse.bacc as bacc
import concourse.mybir as mybir
from concourse import tile
from concourse.bass_utils import run_bass_kernel_spmd

TL = 44
CROP = 2 * TL          # 88
SCALE = 224.0
B, C, IN = 128, 16, 224
NCORES = 8
BPC = B // NCORES      # 16 samples per core
BLK = 8                # samples per chunk -> BLK*C = 128 partitions
NBLK = BPC // BLK      # 2 chunks per core
FREE = CROP * CROP     # 7744
HC = C // 2            # channel half
MAXOFF = IN - CROP     # 136
IMSZ = C * IN * IN
CHSZ = IN * IN
MAXEOFF = (BPC - 1) * IMSZ + MAXOFF * IN + MAXOFF

_nc_cache = {}


def _build_nc():
    nc = bacc.Bacc(None)
    images = nc.declare_dram_parameter(
        "images", [BPC, C, IN, IN], mybir.dt.float32, isOutput=False
    )
    offs = nc.declare_dram_parameter(
        "offs", [1, BPC], mybir.dt.int32, isOutput=False
    )
    maskrow = nc.declare_dram_parameter(
        "maskrow", [1, FREE], mybir.dt.float32, isOutput=False
    )
    ones1 = nc.declare_dram_parameter(
        "ones1", [1, BLK * C], mybir.dt.float32, isOutput=False
    )
    out = nc.declare_dram_parameter(
        "out", [BPC, C, CROP, CROP], mybir.dt.float32, isOutput=True
    )

    with tile.TileContext(nc) as tc:
        with (
            tc.tile_pool(name="const", bufs=1) as cpool,
            tc.tile_pool(name="work", bufs=2) as wpool,
            tc.tile_pool(name="psum", bufs=2, space="PSUM") as ppool,
        ):
            offs_sb = cpool.tile([1, BPC], mybir.dt.int32)
            nc.sync.dma_start(out=offs_sb[:], in_=offs[:])
            # replicate the [1, 7744] mask row to all 128 partitions on-chip
            mrow_sb = cpool.tile([1, FREE], mybir.dt.float32)
            nc.gpsimd.dma_start(out=mrow_sb[:], in_=maskrow[:])
            ones_sb = cpool.tile([1, BLK * C], mybir.dt.float32)
            nc.gpsimd.dma_start(out=ones_sb[:], in_=ones1[:])
            mask_sb = cpool.tile([BLK * C, FREE], mybir.dt.float32)
            pcol = 512
            for ci in range((FREE + pcol - 1) // pcol):
                lo = ci * pcol
                w = min(pcol, FREE - lo)
                pt = ppool.tile([BLK * C, pcol], mybir.dt.float32, tag="pmask")
                nc.tensor.matmul(
                    out=pt[:, 0:w],
                    lhsT=ones_sb[0:1, :],
                    rhs=mrow_sb[0:1, lo : lo + w],
                    start=True,
                    stop=True,
                )
                nc.vector.tensor_copy(out=mask_sb[:, lo : lo + w], in_=pt[:, 0:w])

            regs = {
                "sync": nc.sync.alloc_register("o_reg_sp"),
                "scalar": nc.scalar.alloc_register("o_reg_act"),
            }
            engs = {"sync": nc.sync, "scalar": nc.scalar}
            for blk in range(NBLK):
                t = wpool.tile([BLK * C, FREE], mybir.dt.float32, tag="blk")
                for j in range(BLK):
                    s = blk * BLK + j
                    base = images[s, :, 0:CROP, 0:CROP]
                    hrow = CROP // 2
                    for hi, rk in enumerate(("sync", "scalar")):
                        eng_, reg_ = engs[rk], regs[rk]
                        eng_.reg_load(reg_, offs_sb[0:1, s : s + 1])
                        ov = eng_.snap(
                            reg_, donate=True, min_val=0, max_val=MAXEOFF
                        )
                        src = bass.AP(
                            tensor=base.tensor,
                            offset=ov + hi * hrow * IN,
                            ap=[[CHSZ, C], [IN, hrow], [1, CROP]],
                            dep_tracking_offset=s * IMSZ,
                        )
                        eng_.dma_start(
                            out=t[
                                j * C : (j + 1) * C,
                                hi * hrow * CROP : (hi + 1) * hrow * CROP,
                            ],
                            in_=src,
                        )
                nc.vector.tensor_tensor(
                    out=t[:], in0=t[:], in1=mask_sb[:], op=mybir.AluOpType.mult
                )
                out_view = out[blk * BLK : (blk + 1) * BLK].rearrange(
                    "b c i k -> (b c) (i k)"
                )
                nc.gpsimd.dma_start(out=out_view, in_=t[:])
    nc.finalize()
    return nc


def _get_nc():
    if "nc" not in _nc_cache:
        _nc_cache["nc"] = _build_nc()
    return _nc_cache["nc"]


def _host_offsets(locs):
    locs = np.asarray(locs, dtype=np.float32)
    t = np.clip(locs * np.float32(SCALE), np.float32(TL), np.float32(IN - TL))
    return np.floor(t - np.float32(TL)).astype(np.int32)  # [B, 2] (w, h)


def _host_mask():
    rr = np.arange(CROP, dtype=np.float32)
    sig = lambda z: (1.0 / (1.0 + np.exp(-10.0 * z, dtype=np.float32))).astype(
        np.float32
    )
    prof = sig(rr) - sig(rr - np.float32(CROP))
    mask = np.outer(prof, prof).astype(np.float32).reshape(1, -1)
    return np.ascontiguousarray(mask)


def make_in_maps(images, locs):
    images = np.asarray(images, dtype=np.float32)
    off = _host_offsets(locs)  # [B, 2] (w, h)
    s_idx = np.arange(BPC, dtype=np.int64)
    maskrow = _host_mask()
    ones1 = np.ones((1, BLK * C), dtype=np.float32)
    in_maps = []
    for i in range(NCORES):
        sl = slice(i * BPC, (i + 1) * BPC)
        osh = off[sl].astype(np.int64)
        eoff = (s_idx * IMSZ + osh[:, 0] * IN + osh[:, 1]).astype(np.int32)
        in_maps.append(
            {
                "images": np.ascontiguousarray(images[sl]),
                "offs": np.ascontiguousarray(eoff.reshape(1, -1)),
                "maskrow": maskrow,
                "ones1": ones1,
            }
        )
    return in_maps


def run(images, locs, trace=False, **kwargs):
    nc = _get_nc()
    in_maps = make_in_maps(images, locs)
    res = run_bass_kernel_spmd(
        nc, in_maps, core_ids=list(range(NCORES)), trace=trace, **kwargs
    )
    outs = [np.asarray(res.results[i]["out"]) for i in range(NCORES)]
    full = np.concatenate(outs, axis=0).astype(np.float32)
    return full, res


def kernel(images, locs):
    full, _ = run(images, locs, trace=False)
    return full


# revision 46
# speedup vs baseline: 1.6116x; 1.1240x over previous
"""AttentionCropLayer Trainium2 kernel.

Per sample b: offsets (w,h) = floor(clip(locs[b]*224, 44, 180) - 44); output
out[b] = images[b, :, w:w+88, h:h+88] * mask, with mask the fixed 88x88
sigmoid-profile outer product.

Strategy (pure data parallel, 8 cores x 16 samples):
  - host precomputes per-sample flat element offsets (exact same f32 ops as
    the reference) plus tiny constant tables for the mask
  - device, per chunk of 8 samples (=128 partitions, partition = (sample,
    channel)): per sample two dynamic-offset HWDGE read DMAs (channel halves,
    one per HWDGE ring so every SDMA engine interleaves two descriptor
    streams), each descriptor an exact 352B crop row; per chunk one in-place
    DVE mask-multiply (mask replicated on-chip via an outer-product matmul)
    and one contiguous 3.96MB store
"""

import sys

if "/opt/trn_rl_repo" not in sys.path:
    sys.path.insert(0, "/opt/trn_rl_repo")

import numpy as np

import concourse.bass as bass
import concourse.bacc as bacc
import concourse.mybir as mybir
from concourse import tile
from concourse.bass_utils import run_bass_kernel_spmd

TL = 44
CROP = 2 * TL          # 88
SCALE = 224.0
B, C, IN = 128, 16, 224
NCORES = 8
BPC = B // NCORES      # 16 samples per core
BLK = 8                # samples per chunk -> BLK*C = 128 partitions
NBLK = BPC // BLK      # 2 chunks per core
FREE = CROP * CROP     # 7744
HC = C // 2            # channel half
MAXOFF = IN - CROP     # 136
IMSZ = C * IN * IN
CHSZ = IN * IN
MAXEOFF = (BPC - 1) * IMSZ + MAXOFF * IN + MAXOFF

_nc_cache = {}


def _build_nc():
    nc = bacc.Bacc(None)
    images = nc.declare_dram_parameter(
        "images", [BPC, C, IN, IN], mybir.dt.float32, isOutput=False
    )
    offs = nc.declare_dram_parameter(
        "offs", [1, BPC], mybir.dt.int32, isOutput=False
    )
    maskrow = nc.declare_dram_parameter(
        "maskrow", [1, FREE], mybir.dt.float32, isOutput=False
    )
    ones1 = nc.declare_dram_parameter(
        "ones1", [1, BLK * C], mybir.dt.float32, isOutput=False
    )
    out = nc.declare_dram_parameter(
        "out", [BPC, C, CROP, CROP], mybir.dt.float32, isOutput=True
    )

    with tile.TileContext(nc) as tc:
        with (
            tc.tile_pool(name="const", bufs=1) as cpool,
            tc.tile_pool(name="work", bufs=2) as wpool,
            tc.tile_pool(name="psum", bufs=2, space="PSUM") as ppool,
        ):
            offs_sb = cpool.tile([1, BPC], mybir.dt.int32)
            nc.sync.dma_start(out=offs_sb[:], in_=offs[:])
            # replicate the [1, 7744] mask row to all 128 partitions on-chip
            mrow_sb = cpool.tile([1, FREE], mybir.dt.float32)
            nc.gpsimd.dma_start(out=mrow_sb[:], in_=maskrow[:])
            ones_sb = cpool.tile([1, BLK * C], mybir.dt.float32)
            nc.gpsimd.dma_start(out=ones_sb[:], in_=ones1[:])
            mask_sb = cpool.tile([BLK * C, FREE], mybir.dt.float32)
            pcol = 512
            for ci in range((FREE + pcol - 1) // pcol):
                lo = ci * pcol
                w = min(pcol, FREE - lo)
                pt = ppool.tile([BLK * C, pcol], mybir.dt.float32, tag="pmask")
                nc.tensor.matmul(
                    out=pt[:, 0:w],
                    lhsT=ones_sb[0:1, :],
                    rhs=mrow_sb[0:1, lo : lo + w],
                    start=True,
                    stop=True,
                )
                nc.vector.tensor_copy(out=mask_sb[:, lo : lo + w], in_=pt[:, 0:w])

            regs = {
                "sync": nc.sync.alloc_register("o_reg_sp"),
                "scalar": nc.scalar.alloc_register("o_reg_act"),
            }
            engs = {"sync": nc.sync, "scalar": nc.scalar}
            for blk in range(NBLK):
                t = wpool.tile([BLK * C, FREE], mybir.dt.float32, tag="blk")
                for j in range(BLK):
                    s = blk * BLK + j
                    base = images[s, :, 0:CROP, 0:CROP]
                    rk = "sync" if j % 2 == 0 else "scalar"
                    eng_, reg_ = engs[rk], regs[rk]
                    eng_.reg_load(reg_, offs_sb[0:1, s : s + 1])
                    ov = eng_.snap(reg_, donate=True, min_val=0, max_val=MAXEOFF)
                    src = bass.AP(
                        tensor=base.tensor,
                        offset=ov,
                        ap=[[CHSZ, C], [IN, CROP], [1, CROP]],
                        dep_tracking_offset=s * IMSZ,
                    )
                    eng_.dma_start(out=t[j * C : (j + 1) * C, :], in_=src)
                nc.vector.tensor_tensor(
                    out=t[:], in0=t[:], in1=mask_sb[:], op=mybir.AluOpType.mult
                )
                out_view = out[blk * BLK : (blk + 1) * BLK].rearrange(
                    "b c i k -> (b c) (i k)"
                )
                nc.gpsimd.dma_start(out=out_view, in_=t[:])
    nc.finalize()
    return nc


def _get_nc():
    if "nc" not in _nc_cache:
        _nc_cache["nc"] = _build_nc()
    return _nc_cache["nc"]


def _host_offsets(locs):
    locs = np.asarray(locs, dtype=np.float32)
    t = np.clip(locs * np.float32(SCALE), np.float32(TL), np.float32(IN - TL))
    return np.floor(t - np.float32(TL)).astype(np.int32)  # [B, 2] (w, h)


def _host_mask():
    rr = np.arange(CROP, dtype=np.float32)
    sig = lambda z: (1.0 / (1.0 + np.exp(-10.0 * z, dtype=np.float32))).astype(
        np.float32
    )
    prof = sig(rr) - sig(rr - np.float32(CROP))
    mask = np.outer(prof, prof).astype(np.float32).reshape(1, -1)
    return np.ascontiguousarray(mask)


def make_in_maps(images, locs):
    images = np.asarray(images, dtype=np.float32)
    off = _host_offsets(locs)  # [B, 2] (w, h)
    s_idx = np.arange(BPC, dtype=np.int64)
    maskrow = _host_mask()
    ones1 = np.ones((1, BLK * C), dtype=np.float32)
    in_maps = []
    for i in range(NCORES):
        sl = slice(i * BPC, (i + 1) * BPC)
        osh = off[sl].astype(np.int64)
        eoff = (s_idx * IMSZ + osh[:, 0] * IN + osh[:, 1]).astype(np.int32)
        in_maps.append(
            {
                "images": np.ascontiguousarray(images[sl]),
                "offs": np.ascontiguousarray(eoff.reshape(1, -1)),
                "maskrow": maskrow,
                "ones1": ones1,
            }
        )
    return in_maps


def run(images, locs, trace=False, **kwargs):
    nc = _get_nc()
    in_maps = make_in_maps(images, locs)
    res = run_bass_kernel_spmd(
        nc, in_maps, core_ids=list(range(NCORES)), trace=trace, **kwargs
    )
    outs = [np.asarray(res.results[i]["out"]) for i in range(NCORES)]
    full = np.concatenate(outs, axis=0).astype(np.float32)
    return full, res


def kernel(images, locs):
    full, _ = run(images, locs, trace=False)
    return full
